# revision 2
# baseline (speedup 1.0000x reference)
"""GraphUpsampling kernel for 8x TRN2 NeuronCores.

Math: out = (A / colsum(A)) @ input.reshape(P,C)[descendance]
    == A @ (up / colsum(A)[:,None])          (scale the small side)

Sharding: COLUMN-shard A across 8 cores. Core k owns columns
j in [k*1024, (k+1)*1024). Each core holds the FULL column, so it
computes its own colsum locally -- zero communication. Each core
produces a partial output (8192, 32) = A[:, jk] @ up_scaled[jk]; the
host sums the 8 partials (the unshard reduction).

Device layout: core k's A slice is pre-transposed on host to
at = A[:, jk].T with shape (1024, 8192), so the contraction dim j is
the SBUF partition dim -- no on-chip transpose needed, colsum is a
free-dim vector reduce, and matmul uses at tiles as stationary lhsT.

PSUM: the full partial output (64 i-blocks x [128, 32]) is packed into
4 PSUM banks. A start=True matmul clears has_written bits bank-wide,
so we zero each bank once with a cheap K=1 all-zeros matmul and run
every real matmul with start=False (pure accumulate).
"""

import sys

sys.path.insert(0, "/opt/trn_rl_repo")

import numpy as np

import concourse.bass as bass
import concourse.mybir as mybir
from concourse import bacc
from concourse.bass_utils import run_bass_kernel_spmd
from concourse.tile import TileContext

PARENT = 4096
CHILD = 8192
C = 32
NCORES = 8
JPC = CHILD // NCORES  # 1024 columns of A per core
NSTRIPE = JPC // 128  # 8 stripes of 128 j per core
NIB = CHILD // 128  # 64 i-blocks of 128

_CACHE = {}


def _build_program(repeats=1):
    fp32 = mybir.dt.float32
    nc = bacc.Bacc("TRN2", target_bir_lowering=False)
    at = nc.dram_tensor("at", (JPC, CHILD), fp32, kind="ExternalInput")
    u = nc.dram_tensor("u", (JPC, C), fp32, kind="ExternalInput")
    # Output in scrambled layout [128, 64*32]: y2[p, ib*32+c] = Y[ib*128+p, c].
    # Host unscrambles; this keeps the store DMA contiguous (8KB/partition).
    y2 = nc.dram_tensor("y2", (128, NIB * C), fp32, kind="ExternalOutput")

    with TileContext(nc) as tc:
        with (
            tc.tile_pool(name="stripes", bufs=4) as spool,
            tc.tile_pool(name="small", bufs=1) as small,
            tc.tile_pool(name="uscaled", bufs=NSTRIPE) as upool,
            tc.tile_pool(name="stats", bufs=NSTRIPE) as stpool,
            tc.tile_pool(name="psum", bufs=1, space="PSUM") as ppool,
            tc.tile_pool(name="evict", bufs=1) as epool,
        ):
            zlhs = small.tile([1, 128], fp32, tag="zlhs")
            nc.vector.memset(zlhs, 0.0)
            zrhs = small.tile([1, 512], fp32, tag="zrhs")
            nc.vector.memset(zrhs, 0.0)

            for rep in range(repeats):
                psum_out = ppool.tile([128, NIB * C], fp32)  # 2048 fp32 = 4 banks
                # Zero all 4 banks + set every has_written bit (K=1 matmul).
                for b in range(4):
                    nc.tensor.matmul(
                        psum_out[:, b * 512 : (b + 1) * 512],
                        zlhs[:, :],
                        zrhs[:, :],
                        start=True,
                        stop=False,
                        skip_group_check=True,
                    )

                HALF = CHILD // 2
                for jc in range(NSTRIPE):
                    # Two half-stripes: colsum of half 0 overlaps half 1's DMA.
                    h0 = spool.tile([128, HALF], fp32, tag="h0")
                    nc.sync.dma_start(h0, at[jc * 128 : (jc + 1) * 128, 0:HALF])
                    h1 = spool.tile([128, HALF], fp32, tag="h1")
                    nc.sync.dma_start(h1, at[jc * 128 : (jc + 1) * 128, HALF:CHILD])
                    s0 = stpool.tile([128, 1], fp32, tag="s0")
                    nc.vector.reduce_sum(s0, h0, axis=mybir.AxisListType.X)
                    s1 = stpool.tile([128, 1], fp32, tag="s1")
                    nc.vector.reduce_sum(s1, h1, axis=mybir.AxisListType.X)
                    s = stpool.tile([128, 1], fp32, tag="s")
                    nc.vector.tensor_add(s, s0, s1)
                    r = stpool.tile([128, 1], fp32, tag="r")
                    nc.vector.reciprocal(r, s)
                    uc = upool.tile([128, C], fp32, tag="uc")
                    nc.sync.dma_start(uc, u[jc * 128 : (jc + 1) * 128, :])
                    us = upool.tile([128, C], fp32, tag="us")
                    nc.scalar.activation(
                        us, uc, mybir.ActivationFunctionType.Copy, scale=r
                    )
                    last = jc == NSTRIPE - 1
                    # outT[c, i] packed: i-chunk q (512 wide) -> bank b=q//4,
                    # col-group g=q%4 at psum partitions [32g, 32g+32).
                    # us is stationary (32 cols), at-stripe chunks are moving
                    # (N=512) -- avoids a 128-col LDWEIGHTS per matmul.
                    for q in range(CHILD // 512):
                        b, g = divmod(q, 4)
                        half = h0 if q < 8 else h1
                        off = q * 512 if q < 8 else q * 512 - HALF
                        nc.tensor.matmul(
                            psum_out[32 * g : 32 * (g + 1), b * 512 : (b + 1) * 512],
                            us[:, :],
                            half[:, off : off + 512],
                            start=False,
                            stop=last,
                            skip_group_check=True,
                            tile_position=(0, 32 * g),
                        )

                out_sb = epool.tile([128, NIB * C], fp32)
                for b in range(4):
                    nc.vector.tensor_copy(
                        out_sb[:, b * 512 : (b + 1) * 512],
                        psum_out[:, b * 512 : (b + 1) * 512],
                    )
                nc.sync.dma_start(y2[:, :], out_sb)

    nc.finalize()
    return nc


def make_in_maps(input, A, descendance):
    input = np.asarray(input)
    A = np.asarray(A, dtype=np.float32)
    desc = np.asarray(descendance).astype(np.int64)

    matrix_in = np.ascontiguousarray(input, dtype=np.float32).reshape(PARENT, C)
    up = matrix_in[desc]  # (CHILD, C) gather

    # Shard: core k gets at = A[:, k*JPC:(k+1)*JPC].T  (contiguous (JPC, CHILD))
    at_all = np.ascontiguousarray(
        A.reshape(CHILD, NCORES, JPC).transpose(1, 2, 0)
    )  # (NCORES, JPC, CHILD)
    in_maps = []
    for k in range(NCORES):
        in_maps.append(
            {
                "at": at_all[k],
                "u": np.ascontiguousarray(up[k * JPC : (k + 1) * JPC]),
            }
        )
    return in_maps


def kernel(input, A, descendance):
    if "nc" not in _CACHE:
        _CACHE["nc"] = _build_program()
    nc = _CACHE["nc"]

    in_maps = make_in_maps(input, A, descendance)

    res = run_bass_kernel_spmd(nc, in_maps, core_ids=list(range(NCORES)))
    outs = res.results

    acc = np.zeros((128, NIB * C), dtype=np.float64)
    for k in range(NCORES):
        acc += outs[k]["y2"]
    # Unscramble: y2[32g+c, 512b+o] -> Y[(4b+g)*512+o, c]
    Y = (
        acc.reshape(4, C, 4, 512)
        .transpose(2, 0, 3, 1)
        .reshape(CHILD, C)
        .astype(np.float32)
    )
    return Y.reshape(1, C, CHILD)



# revision 3
# speedup vs baseline: 162.3559x; 162.3559x over previous
"""GraphUpsampling kernel for 8x TRN2 NeuronCores.

Math: out = (A / colsum(A)) @ input.reshape(P,C)[descendance]
    == A @ (up / colsum(A)[:,None])          (scale the small side)

Sharding: COLUMN-shard A across 8 cores. Core k owns columns
j in [k*1024, (k+1)*1024). Each core holds the FULL column, so it
computes its own colsum locally -- zero communication. Each core
produces a partial output (8192, 32) = A[:, jk] @ up_scaled[jk]; the
host sums the 8 partials (the unshard reduction).

Precision: A is cast to bf16 on host during the shard/transpose prep.
This halves HBM traffic (the memory-regime bottleneck) and runs the
PE at 1 cycle/row instead of fp32's 4. Output error ~1.5e-3 l2, far
inside the 2e-2 gate. colsum is computed on-device from the bf16 data
(fp32 accumulate), so normalization is self-consistent.

Device layout: core k's A slice is pre-transposed on host to
at = A[:, jk].T with shape (1024, 8192), so the contraction dim j is
the SBUF partition dim -- no on-chip transpose needed, colsum is a
free-dim vector reduce, and matmul uses at tiles as moving data.

PSUM: the full partial output (64 i-blocks x [128, 32]) is packed into
4 PSUM banks. A start=True matmul clears has_written bits bank-wide,
so we zero each bank once with a cheap K=1 all-zeros matmul and run
every real matmul with start=False (pure accumulate).
"""

import sys

sys.path.insert(0, "/opt/trn_rl_repo")

import numpy as np
import ml_dtypes

import concourse.bass as bass
import concourse.mybir as mybir
from concourse import bacc
from concourse.bass_utils import run_bass_kernel_spmd
from concourse.tile import TileContext

PARENT = 4096
CHILD = 8192
C = 32
NCORES = 8
JPC = CHILD // NCORES  # 1024 columns of A per core
NSTRIPE = JPC // 128  # 8 stripes of 128 j per core
NIB = CHILD // 128  # 64 i-blocks of 128

_CACHE = {}


def _build_program(repeats=1):
    fp32 = mybir.dt.float32
    bf16 = mybir.dt.bfloat16
    nc = bacc.Bacc("TRN2", target_bir_lowering=False)
    at = nc.dram_tensor("at", (JPC, CHILD), bf16, kind="ExternalInput")
    u = nc.dram_tensor("u", (JPC, C), bf16, kind="ExternalInput")
    # Output in scrambled layout [128, 64*32]: y2[p, ib*32+c] = Y[ib*128+p, c].
    # Host unscrambles; this keeps the store DMA contiguous (8KB/partition).
    y2 = nc.dram_tensor("y2", (128, NIB * C), fp32, kind="ExternalOutput")

    with TileContext(nc) as tc:
        with (
            tc.tile_pool(name="stripes", bufs=4) as spool,
            tc.tile_pool(name="small", bufs=1) as small,
            tc.tile_pool(name="uscaled", bufs=NSTRIPE) as upool,
            tc.tile_pool(name="stats", bufs=NSTRIPE) as stpool,
            tc.tile_pool(name="psum", bufs=1, space="PSUM") as ppool,
            tc.tile_pool(name="evict", bufs=1) as epool,
        ):
            zlhs = small.tile([1, 128], bf16, tag="zlhs")
            nc.vector.memset(zlhs, 0.0)
            zrhs = small.tile([1, 512], bf16, tag="zrhs")
            nc.vector.memset(zrhs, 0.0)

            for rep in range(repeats):
                psum_out = ppool.tile([128, NIB * C], fp32)  # 2048 fp32 = 4 banks
                # Zero all 4 banks + set every has_written bit (K=1 matmul).
                for b in range(4):
                    nc.tensor.matmul(
                        psum_out[:, b * 512 : (b + 1) * 512],
                        zlhs[:, :],
                        zrhs[:, :],
                        start=True,
                        stop=False,
                        skip_group_check=True,
                    )

                HALF = CHILD // 2
                for jc in range(NSTRIPE):
                    # Two half-stripes: colsum of half 0 overlaps half 1's DMA.
                    h0 = spool.tile([128, HALF], bf16, tag="h0")
                    nc.sync.dma_start(h0, at[jc * 128 : (jc + 1) * 128, 0:HALF])
                    h1 = spool.tile([128, HALF], bf16, tag="h1")
                    nc.sync.dma_start(h1, at[jc * 128 : (jc + 1) * 128, HALF:CHILD])
                    s0 = stpool.tile([128, 1], fp32, tag="s0")
                    nc.vector.reduce_sum(s0, h0, axis=mybir.AxisListType.X)
                    s1 = stpool.tile([128, 1], fp32, tag="s1")
                    nc.vector.reduce_sum(s1, h1, axis=mybir.AxisListType.X)
                    s = stpool.tile([128, 1], fp32, tag="s")
                    nc.vector.tensor_add(s, s0, s1)
                    r = stpool.tile([128, 1], fp32, tag="r")
                    nc.vector.reciprocal(r, s)
                    uc = upool.tile([128, C], bf16, tag="uc")
                    nc.sync.dma_start(uc, u[jc * 128 : (jc + 1) * 128, :])
                    us = upool.tile([128, C], bf16, tag="us")
                    nc.scalar.activation(
                        us, uc, mybir.ActivationFunctionType.Copy, scale=r
                    )
                    last = jc == NSTRIPE - 1
                    # outT[c, i] packed: i-chunk q (512 wide) -> bank b=q//4,
                    # col-group g=q%4 at psum partitions [32g, 32g+32).
                    # us is stationary (32 cols), at-stripe chunks are moving
                    # (N=512) -- avoids a 128-col LDWEIGHTS per matmul.
                    for q in range(CHILD // 512):
                        b, g = divmod(q, 4)
                        half = h0 if q < 8 else h1
                        off = q * 512 if q < 8 else q * 512 - HALF
                        nc.tensor.matmul(
                            psum_out[32 * g : 32 * (g + 1), b * 512 : (b + 1) * 512],
                            us[:, :],
                            half[:, off : off + 512],
                            start=False,
                            stop=last,
                            skip_group_check=True,
                            tile_position=(0, 32 * g),
                        )

                out_sb = epool.tile([128, NIB * C], fp32)
                for b in range(4):
                    nc.vector.tensor_copy(
                        out_sb[:, b * 512 : (b + 1) * 512],
                        psum_out[:, b * 512 : (b + 1) * 512],
                    )
                nc.sync.dma_start(y2[:, :], out_sb)

    nc.finalize()
    return nc


def make_in_maps(input, A, descendance):
    input = np.asarray(input)
    A = np.asarray(A, dtype=np.float32)
    desc = np.asarray(descendance).astype(np.int64)

    matrix_in = np.ascontiguousarray(input, dtype=np.float32).reshape(PARENT, C)
    up = matrix_in[desc].astype(ml_dtypes.bfloat16)  # (CHILD, C) gather

    # Shard: core k gets at = A[:, k*JPC:(k+1)*JPC].T  (contiguous (JPC, CHILD))
    A16 = A.astype(ml_dtypes.bfloat16)
    at_all = np.ascontiguousarray(
        A16.reshape(CHILD, NCORES, JPC).transpose(1, 2, 0)
    )  # (NCORES, JPC, CHILD)
    in_maps = []
    for k in range(NCORES):
        in_maps.append(
            {
                "at": at_all[k],
                "u": np.ascontiguousarray(up[k * JPC : (k + 1) * JPC]),
            }
        )
    return in_maps


def kernel(input, A, descendance):
    if "nc" not in _CACHE:
        _CACHE["nc"] = _build_program()
    nc = _CACHE["nc"]

    in_maps = make_in_maps(input, A, descendance)

    res = run_bass_kernel_spmd(nc, in_maps, core_ids=list(range(NCORES)))
    outs = res.results

    acc = np.zeros((128, NIB * C), dtype=np.float64)
    for k in range(NCORES):
        acc += outs[k]["y2"]
    # Unscramble: y2[32g+c, 512b+o] -> Y[(4b+g)*512+o, c]
    Y = (
        acc.reshape(4, C, 4, 512)
        .transpose(2, 0, 3, 1)
        .reshape(CHILD, C)
        .astype(np.float32)
    )
    return Y.reshape(1, C, CHILD)


# revision 18
# speedup vs baseline: 400.1298x; 2.4645x over previous
"""GraphUpsampling kernel for 8x TRN2 NeuronCores.

Math: out = (A / colsum(A)) @ input.reshape(P,C)[descendance]
    == A @ us,   us = up / colsum(A)[:,None]   (scale the small side)

Sharding: COLUMN-shard A across 8 cores. Core k owns columns
j in [k*1024, (k+1)*1024). Each core produces a partial output
(8192, 32) = A[:, jk] @ us[jk]; the host sums the 8 partials.

Memory-regime optimization: A dominates traffic (256MB fp32). The
default mode ships A as fp8_e4m3 of the CENTERED matrix
B = 2*(A - 0.5) (A is uniform [0,1)): A = 0.5*ones + B/2, so
  out = (B @ (us/2)) + 0.5 * ones @ us
The device computes P = B_fp8 @ bf16(us/2) (mixed-dtype matmul:
bf16 stationary x fp8 moving, 1 cycle/row); the rank-1 ones-term is
exact and added on the host (it is a single [C] vector). This cuts
HBM traffic 4x vs fp32 and keeps l2 error ~1.2e-2 (gate 2e-2).
us is pre-normalized on host (colsum in exact fp32).

Device layout: core k's A slice is pre-transposed on host to
at = B[:, jk].T with shape (1024, 8192): contraction dim j on SBUF
partitions, so matmul streams at tiles as moving data.

PSUM: the partial output (64 i-blocks x [128, 32]) is packed into
4 PSUM banks, double-buffered across reps. A start=True matmul
clears has_written bank-wide, so banks are zeroed once per rep by a
cheap K=1 all-zeros matmul and all real matmuls accumulate.

MODE selects the precision/colsum scheme:
  fp8_hosted  -- the above (default)
  bf16_hosted -- A in bf16, us pre-normalized on host
  bf16_tree   -- A in bf16, colsum on device (DVE fold + ACT accum)
"""

import sys

sys.path.insert(0, "/opt/trn_rl_repo")

import numpy as np
import ml_dtypes

import concourse.bass as bass
import concourse.mybir as mybir
from concourse import bacc
from concourse.bass_utils import run_bass_kernel_spmd
from concourse.tile import TileContext

PARENT = 4096
CHILD = 8192
C = 32
NCORES = 8
JPC = CHILD // NCORES  # 1024 columns of A per core
NSTRIPE = JPC // 128  # 8 stripes of 128 j per core
NIB = CHILD // 128  # 64 i-blocks of 128

MODE = "fp8_hosted"

_CACHE = {}


def _build_program(repeats=1, mode=None):
    if mode is None:
        mode = MODE
    fp32 = mybir.dt.float32
    bf16 = mybir.dt.bfloat16
    fp8 = mybir.dt.float8e4
    adt = fp8 if mode == "fp8_hosted" else bf16
    hosted = mode != "bf16_tree"

    nc = bacc.Bacc("TRN2", target_bir_lowering=False)
    at = nc.dram_tensor("at", (JPC, CHILD), adt, kind="ExternalInput")
    # u packed on host as [128, NSTRIPE*C]: column block jc holds
    # us[jc*128:(jc+1)*128, :] -- one DMA per rep.
    u = nc.dram_tensor("u", (128, NSTRIPE * C), bf16, kind="ExternalInput")
    # Output in scrambled layout [128, 64*32]: y2[p, ib*32+c] = Y[ib*128+p, c].
    # Host unscrambles; keeps the store DMA contiguous.
    y2 = nc.dram_tensor("y2", (128, NIB * C), fp32, kind="ExternalOutput")

    HALF = CHILD // 2
    with TileContext(nc) as tc:
        with (
            tc.tile_pool(name="stripes", bufs=8) as spool,
            tc.tile_pool(name="small", bufs=1) as small,
            tc.tile_pool(name="uscaled", bufs=3) as upool,
            tc.tile_pool(name="scratch", bufs=3) as scpool,
            tc.tile_pool(name="stats", bufs=NSTRIPE) as stpool,
            tc.tile_pool(name="psum", bufs=2, space="PSUM") as ppool,
            tc.tile_pool(name="evict", bufs=2) as epool,
        ):
            zlhs = small.tile([1, 128], adt, tag="zlhs")
            nc.vector.memset(zlhs, 0.0)
            zrhs = small.tile([1, 512], adt, tag="zrhs")
            nc.vector.memset(zrhs, 0.0)

            for rep in range(repeats):
                psum_out = ppool.tile([128, NIB * C], fp32)  # 2048 fp32 = 4 banks
                for b in range(4):
                    nc.tensor.matmul(
                        psum_out[:, b * 512 : (b + 1) * 512],
                        zlhs[:, :],
                        zrhs[:, :],
                        start=True,
                        stop=False,
                        skip_group_check=True,
                    )

                uc = upool.tile([128, NSTRIPE * C], bf16, tag="uc")
                nc.sync.dma_start(uc, u[:, :])

                for jc in range(NSTRIPE):
                    if hosted:
                        # One full-stripe DMA; no on-device colsum.
                        h = spool.tile([128, CHILD], adt, tag="h")
                        nc.sync.dma_start(h, at[jc * 128 : (jc + 1) * 128, :])
                        h0 = h[:, 0:HALF]
                        h1 = h[:, HALF:CHILD]
                        us = uc[:, jc * C : (jc + 1) * C]
                    else:
                        # Two half-stripe DMAs; colsum of the folded halves.
                        h0 = spool.tile([128, HALF], adt, tag="h0")
                        nc.sync.dma_start(h0, at[jc * 128 : (jc + 1) * 128, 0:HALF])
                        h1 = spool.tile([128, HALF], adt, tag="h1")
                        nc.sync.dma_start(h1, at[jc * 128 : (jc + 1) * 128, HALF:])
                        t1 = scpool.tile([128, HALF], adt, tag="t1")
                        nc.vector.tensor_add(t1, h0, h1)
                        s = stpool.tile([128, 1], fp32, tag="s")
                        hs0 = scpool.tile([128, HALF], adt, tag="hs0")
                        nc.scalar.activation(
                            hs0,
                            t1,
                            mybir.ActivationFunctionType.Copy,
                            accum_out=s,
                        )
                        r = stpool.tile([128, 1], fp32, tag="r")
                        nc.vector.reciprocal(r, s)
                        us = upool.tile([128, C], bf16, tag="us")
                        nc.scalar.activation(
                            us,
                            uc[:, jc * C : (jc + 1) * C],
                            mybir.ActivationFunctionType.Copy,
                            scale=r,
                        )

                    last = jc == NSTRIPE - 1
                    # outT[c, i] packed: i-chunk q (512 wide) -> bank b=q//4,
                    # col-group g=q%4 at psum partitions [32g, 32g+32).
                    # us is stationary (32 cols); at chunks are moving.
                    for q in range(CHILD // 512):
                        b, g = divmod(q, 4)
                        half = h0 if q < 8 else h1
                        off = q * 512 if q < 8 else q * 512 - HALF
                        nc.tensor.matmul(
                            psum_out[32 * g : 32 * (g + 1), b * 512 : (b + 1) * 512],
                            us[:, :],
                            half[:, off : off + 512],
                            start=False,
                            stop=last,
                            skip_group_check=True,
                            tile_position=(0, 32 * g),
                        )

                out_sb = epool.tile([128, NIB * C], fp32)
                nc.vector.tensor_copy(out_sb, psum_out)
                nc.scalar.dma_start(y2[:, :], out_sb)

    nc.finalize()
    return nc


def make_in_maps(input, A, descendance):
    """Host-side shard prep. Returns (in_maps, corr) where corr is the
    [C] ones-term for fp8_hosted (zeros otherwise)."""
    input = np.asarray(input)
    A = np.asarray(A, dtype=np.float32)
    desc = np.asarray(descendance).astype(np.int64)

    matrix_in = np.ascontiguousarray(input, dtype=np.float32).reshape(PARENT, C)
    up32 = matrix_in[desc]  # (CHILD, C) gather

    corr = np.zeros((C,), dtype=np.float64)
    if MODE in ("fp8_hosted", "bf16_hosted"):
        us32 = up32 / A.sum(axis=0, dtype=np.float64).astype(np.float32)[:, None]
    else:
        us32 = up32

    if MODE == "fp8_hosted":
        ush = (us32 * 0.5).astype(ml_dtypes.bfloat16)  # device stationary
        corr = ush.astype(np.float64).sum(axis=0)  # = 0.5 * sum(us)
        B = (A - 0.5) * 2.0
        at_all = np.ascontiguousarray(
            B.reshape(CHILD, NCORES, JPC).transpose(1, 2, 0)
        ).astype(ml_dtypes.float8_e4m3fn)
        uh = ush
    else:
        at_all = np.ascontiguousarray(
            A.astype(ml_dtypes.bfloat16).reshape(CHILD, NCORES, JPC).transpose(1, 2, 0)
        )
        uh = us32.astype(ml_dtypes.bfloat16)

    in_maps = []
    for k in range(NCORES):
        uk = uh[k * JPC : (k + 1) * JPC]  # (1024, 32)
        u2 = np.ascontiguousarray(
            uk.reshape(NSTRIPE, 128, C).transpose(1, 0, 2).reshape(128, NSTRIPE * C)
        )
        in_maps.append({"at": at_all[k], "u": u2})
    return in_maps, corr


def kernel(input, A, descendance):
    if "nc" not in _CACHE:
        _CACHE["nc"] = _build_program()
    nc = _CACHE["nc"]

    in_maps, corr = make_in_maps(input, A, descendance)

    res = run_bass_kernel_spmd(nc, in_maps, core_ids=list(range(NCORES)))
    outs = res.results

    acc = np.zeros((128, NIB * C), dtype=np.float64)
    for k in range(NCORES):
        acc += outs[k]["y2"].astype(np.float64)
    # Unscramble: y2[32g+c, 512b+o] -> Y[(4b+g)*512+o, c]
    Y = acc.reshape(4, C, 4, 512).transpose(2, 0, 3, 1).reshape(CHILD, C)
    Y = (Y + corr[None, :]).astype(np.float32)
    return Y.reshape(1, C, CHILD)


# revision 24
# speedup vs baseline: 707.1766x; 1.7674x over previous
"""GraphUpsampling kernel for 8x TRN2 NeuronCores.

Math: out = (A / colsum(A)) @ input.reshape(P,C)[descendance]
    == A @ us,   us = up / colsum(A)[:,None]   (scale the small side)

Sharding: COLUMN-shard A across 8 cores. Core k owns columns
j in [k*1024, (k+1)*1024). Each core produces a partial output
(8192, 32) = A[:, jk] @ us[jk]; the host sums the 8 partials.

Memory-regime optimization: A dominates traffic (256MB fp32). The
default mode ships A as fp8_e4m3 of the CENTERED matrix
B = 2*(A - 0.5) (A is uniform [0,1)): A = 0.5*ones + B/2, so
  out = (B @ (us/2)) + 0.5 * ones @ us
The device computes P = B_fp8 @ bf16(us/2) (mixed-dtype matmul:
bf16 stationary x fp8 moving, 1 cycle/row); the rank-1 ones-term is
exact and added on the host (it is a single [C] vector). This cuts
HBM traffic 4x vs fp32 and keeps l2 error ~1.2e-2 (gate 2e-2).
us is pre-normalized on host (colsum in exact fp32).

Device layout: core k's A slice is pre-transposed on host to
at = B[:, jk].T with shape (1024, 8192): contraction dim j on SBUF
partitions, so matmul streams at tiles as moving data.

PSUM: the partial output (64 i-blocks x [128, 32]) is packed into
4 PSUM banks, double-buffered across reps. A start=True matmul
clears has_written bank-wide, so banks are zeroed once per rep by a
cheap K=1 all-zeros matmul and all real matmuls accumulate.

MODE selects the precision/colsum scheme:
  fp8_hosted  -- the above (default)
  bf16_hosted -- A in bf16, us pre-normalized on host
  bf16_tree   -- A in bf16, colsum on device (DVE fold + ACT accum)
"""

import sys

sys.path.insert(0, "/opt/trn_rl_repo")

import numpy as np
import ml_dtypes

import concourse.bass as bass
import concourse.mybir as mybir
from concourse import bacc
from concourse.bass_utils import run_bass_kernel_spmd
from concourse.tile import TileContext

PARENT = 4096
CHILD = 8192
C = 32
NCORES = 8
JPC = CHILD // NCORES  # 1024 columns of A per core
NSTRIPE = JPC // 128  # 8 stripes of 128 j per core
NIB = CHILD // 128  # 64 i-blocks of 128

MODE = "fp8_hosted"

_CACHE = {}

MM_ORDER = "pairs"


def _mm_order():
    '''Issue order of the 16 per-stripe matmul chunks q (bank b=q//4,
    group g=q%4).'''
    if MM_ORDER == "bmajor":
        return list(range(16))
    if MM_ORDER == "diag":
        # consecutive mms change both g and b
        return [4 * ((g + w) % 4) + g for w in range(4) for g in range(4)]
    if MM_ORDER == "pairs":
        # runs of 2 with the same group g (one LDWEIGHTS reuse),
        # banks alternate within and across pairs
        order = []
        for w in range(2):
            for g in range(4):
                b0 = (g + 2 * w) % 4
                b1 = (g + 2 * w + 1) % 4
                order += [4 * b0 + g, 4 * b1 + g]
        return order
    raise ValueError(MM_ORDER)


def _build_program(repeats=1, mode=None):
    if mode is None:
        mode = MODE
    fp32 = mybir.dt.float32
    bf16 = mybir.dt.bfloat16
    fp8 = mybir.dt.float8e4
    adt = fp8 if mode == "fp8_hosted" else bf16
    hosted = mode != "bf16_tree"

    nc = bacc.Bacc("TRN2", target_bir_lowering=False)
    at = nc.dram_tensor("at", (JPC, CHILD), adt, kind="ExternalInput")
    # u packed on host as [128, NSTRIPE*C]: column block jc holds
    # us[jc*128:(jc+1)*128, :] -- one DMA per rep.
    u = nc.dram_tensor("u", (128, NSTRIPE * C), bf16, kind="ExternalInput")
    # Output in scrambled layout [128, 64*32]: y2[p, ib*32+c] = Y[ib*128+p, c].
    # Host unscrambles; keeps the store DMA contiguous.
    y2 = nc.dram_tensor("y2", (128, NIB * C), fp32, kind="ExternalOutput")

    HALF = CHILD // 2
    with TileContext(nc) as tc:
        with (
            tc.tile_pool(name="stripes", bufs=8) as spool,
            tc.tile_pool(name="small", bufs=1) as small,
            tc.tile_pool(name="uscaled", bufs=3) as upool,
            tc.tile_pool(name="scratch", bufs=3) as scpool,
            tc.tile_pool(name="stats", bufs=NSTRIPE) as stpool,
            tc.tile_pool(name="psum", bufs=2, space="PSUM") as ppool,
            tc.tile_pool(name="evict", bufs=2) as epool,
        ):
            zlhs = small.tile([1, 128], adt, tag="zlhs")
            nc.vector.memset(zlhs, 0.0)
            zrhs = small.tile([1, 512], adt, tag="zrhs")
            nc.vector.memset(zrhs, 0.0)

            for rep in range(repeats):
                psum_out = ppool.tile([128, NIB * C], fp32)  # 2048 fp32 = 4 banks
                for b in range(4):
                    nc.tensor.matmul(
                        psum_out[:, b * 512 : (b + 1) * 512],
                        zlhs[:, :],
                        zrhs[:, :],
                        start=True,
                        stop=False,
                        skip_group_check=True,
                    )

                uc = upool.tile([128, NSTRIPE * C], bf16, tag="uc")
                nc.sync.dma_start(uc, u[:, :])

                for jc in range(NSTRIPE):
                    if hosted:
                        # One full-stripe DMA; no on-device colsum. Early
                        # stripes issue from SP, late ones from ACT: two
                        # HWDGE issuers overlap their transfer streams.
                        eng = nc.sync if jc < 4 else nc.scalar
                        h = spool.tile([128, CHILD], adt, tag="h")
                        eng.dma_start(h, at[jc * 128 : (jc + 1) * 128, :])
                        h0 = h[:, 0:HALF]
                        h1 = h[:, HALF:CHILD]
                        us = uc[:, jc * C : (jc + 1) * C]
                    else:
                        # Two half-stripe DMAs; colsum of the folded halves.
                        h0 = spool.tile([128, HALF], adt, tag="h0")
                        nc.sync.dma_start(h0, at[jc * 128 : (jc + 1) * 128, 0:HALF])
                        h1 = spool.tile([128, HALF], adt, tag="h1")
                        nc.sync.dma_start(h1, at[jc * 128 : (jc + 1) * 128, HALF:])
                        t1 = scpool.tile([128, HALF], adt, tag="t1")
                        nc.vector.tensor_add(t1, h0, h1)
                        s = stpool.tile([128, 1], fp32, tag="s")
                        hs0 = scpool.tile([128, HALF], adt, tag="hs0")
                        nc.scalar.activation(
                            hs0,
                            t1,
                            mybir.ActivationFunctionType.Copy,
                            accum_out=s,
                        )
                        r = stpool.tile([128, 1], fp32, tag="r")
                        nc.vector.reciprocal(r, s)
                        us = upool.tile([128, C], bf16, tag="us")
                        nc.scalar.activation(
                            us,
                            uc[:, jc * C : (jc + 1) * C],
                            mybir.ActivationFunctionType.Copy,
                            scale=r,
                        )

                    last = jc == NSTRIPE - 1
                    # outT[c, i] packed: i-chunk q (512 wide) -> bank b=q//4,
                    # col-group g=q%4 at psum partitions [32g, 32g+32).
                    # us is stationary (32 cols); at chunks are moving.
                    # Matmul issue order (see _mm_order): avoids
                    # back-to-back writes into one PSUM bank.
                    for q in _mm_order():
                        b, g = divmod(q, 4)
                        half = h0 if q < 8 else h1
                        off = q * 512 if q < 8 else q * 512 - HALF
                        nc.tensor.matmul(
                            psum_out[32 * g : 32 * (g + 1), b * 512 : (b + 1) * 512],
                            us[:, :],
                            half[:, off : off + 512],
                            start=False,
                            stop=last,
                            skip_group_check=True,
                            tile_position=(0, 32 * g),
                        )

                out_sb = epool.tile([128, NIB * C], fp32)
                nc.vector.tensor_copy(out_sb, psum_out)
                nc.scalar.dma_start(y2[:, :], out_sb)

    nc.finalize()
    return nc


def make_in_maps(input, A, descendance):
    """Host-side shard prep. Returns (in_maps, corr) where corr is the
    [C] ones-term for fp8_hosted (zeros otherwise)."""
    input = np.asarray(input)
    A = np.asarray(A, dtype=np.float32)
    desc = np.asarray(descendance).astype(np.int64)

    matrix_in = np.ascontiguousarray(input, dtype=np.float32).reshape(PARENT, C)
    up32 = matrix_in[desc]  # (CHILD, C) gather

    corr = np.zeros((C,), dtype=np.float64)
    if MODE in ("fp8_hosted", "bf16_hosted"):
        us32 = up32 / A.sum(axis=0, dtype=np.float64).astype(np.float32)[:, None]
    else:
        us32 = up32

    if MODE == "fp8_hosted":
        ush = (us32 * 0.5).astype(ml_dtypes.bfloat16)  # device stationary
        corr = ush.astype(np.float64).sum(axis=0)  # = 0.5 * sum(us)
        B = (A - 0.5) * 2.0
        at_all = np.ascontiguousarray(
            B.reshape(CHILD, NCORES, JPC).transpose(1, 2, 0)
        ).astype(ml_dtypes.float8_e4m3fn)
        uh = ush
    else:
        at_all = np.ascontiguousarray(
            A.astype(ml_dtypes.bfloat16).reshape(CHILD, NCORES, JPC).transpose(1, 2, 0)
        )
        uh = us32.astype(ml_dtypes.bfloat16)

    in_maps = []
    for k in range(NCORES):
        uk = uh[k * JPC : (k + 1) * JPC]  # (1024, 32)
        u2 = np.ascontiguousarray(
            uk.reshape(NSTRIPE, 128, C).transpose(1, 0, 2).reshape(128, NSTRIPE * C)
        )
        in_maps.append({"at": at_all[k], "u": u2})
    return in_maps, corr


def kernel(input, A, descendance):
    if "nc" not in _CACHE:
        _CACHE["nc"] = _build_program()
    nc = _CACHE["nc"]

    in_maps, corr = make_in_maps(input, A, descendance)

    res = run_bass_kernel_spmd(nc, in_maps, core_ids=list(range(NCORES)))
    outs = res.results

    acc = np.zeros((128, NIB * C), dtype=np.float64)
    for k in range(NCORES):
        acc += outs[k]["y2"].astype(np.float64)
    # Unscramble: y2[32g+c, 512b+o] -> Y[(4b+g)*512+o, c]
    Y = acc.reshape(4, C, 4, 512).transpose(2, 0, 3, 1).reshape(CHILD, C)
    Y = (Y + corr[None, :]).astype(np.float32)
    return Y.reshape(1, C, CHILD)
